# revision 16
# baseline (speedup 1.0000x reference)
"""Trainium2 Bass kernel for NeuralTensorLayer (order-1/2/3 polynomial layer).

    out[b,l] = bias[l] + sum_i X[b,i] W1[i,l]
             + sum_ij X[b,i] X[b,j] W2[i,j,l]
             + sum_ijk X[b,i] X[b,j] X[b,k] W3[i,j,k,l]

with B=32768, D=K=32, data-parallel over 8 NeuronCores (4096 rows each).

Strategy (per core):
  * Exploit (i,j) symmetry: only the 528 pairs i<=j are needed against
    host-symmetrized weights W3s[ij,k,l] = W3[i,j,k,l]+W3[j,i,k,l] (i<j),
    cutting the dominant matmul contraction from 1024 -> 528 (+32 X rows).
  * Per 512-row supertile: load host-pretransposed X^T (bf16), expand its
    rows to pair rows with two constant 0/1 selection matmuls on the PE
    (exp[p,b]=X[i_p,b], rep[p,b]=X[j_p,b]), multiply on the DVE to get
    Z^T[p,b] = X_i X_j in bf16.  Contraction chunks: 4x128 pairs + a 48-row
    chunk holding 16 pairs plus X^T itself (for the order-1 term).
  * One fused matmul group (bf16, fp32 PSUM accumulation) per 128-row tile:
    T[b, l*34+k] = sum_p Z^T[p,b] Wcat[p, l*34+k], where k<32 are the
    order-3 T3 columns, k=32 is out_low (W2s rows + W1 on the X rows), and
    k=33 is zero padding (keeps the DVE post-multiply 4B-aligned at 2x).
  * Post: stage T to SBUF bf16 (ScalarE), U = T * Xext broadcast (DVE 2x,
    Xext host-padded with [1,0] cols), reduce over k=34 -> out (DVE).
    bias added on host.
"""

import numpy as np
import ml_dtypes
from contextlib import ExitStack

import concourse.bass as bass
import concourse.bacc as bacc
import concourse.tile as tile
from concourse import mybir
from concourse import bass_utils

# Drop redundant LDWEIGHTS from the BIR before walrus codegen: matmuls that
# share a stationary operand (the three N-splits per contraction chunk)
# each carry their own Ldweights (~107ns x ~550 loads, walrus's ldw-opt pass
# is disabled/broken).  A load is elided when the previous PE weight-op in
# SCHEDULED order has a byte-identical weight AP and the load itself carries
# no semaphore waits/updates (so the PE weight registers provably still hold
# the same data and no sync edge is lost).
def _dedup_ldweights(bir_json: bytes) -> bytes:
    import json as _json

    d = _json.loads(bir_json)
    dropped = 0
    for fn in d.get("functions", []):
        for blk in fn.get("blocks", []):
            out = []
            last = None
            for i in blk.get("instructions", []):
                if i.get("engine") == "PE" and i.get("opcode") in ("Ldweights", "Matmult"):
                    w = i["ins"][-1] if i["opcode"] == "Matmult" else i["ins"][0]
                    key = (w.get("memref"), w.get("offset"), _json.dumps(w.get("ap")),
                           w.get("dtype"), _json.dumps(i.get("tile_position")),
                           _json.dumps(i.get("tile_size")), i.get("perf_mode"))
                    if i["opcode"] == "Ldweights":
                        si = i.get("sync_info") or {}
                        if (key == last and not si.get("on_wait")
                                and not si.get("on_update")):
                            dropped += 1
                            continue
                        last = key
                elif i.get("engine") == "PE":
                    last = None  # unknown PE op: invalidate weight-reuse state
                out.append(i)
            blk["instructions"] = out
    return _json.dumps(d).encode()


if not getattr(bass_utils, "_ldw_dedup_patched", False):
    _orig_compile_bir_kernel = bass_utils.compile_bir_kernel

    def _compile_bir_kernel_dedup(bir_json, tmpdir, neff_name="file.neff"):
        return _orig_compile_bir_kernel(_dedup_ldweights(bir_json), tmpdir, neff_name)

    bass_utils.compile_bir_kernel = _compile_bir_kernel_dedup
    import concourse.bass2jax as _b2j

    _b2j.compile_bir_kernel = _compile_bir_kernel_dedup
    bass_utils._ldw_dedup_patched = True

BF16 = ml_dtypes.bfloat16

B, D, KOUT = 32768, 32, 32
NCORES = 8
BLOC = B // NCORES          # 4096 rows per core
P = 128                     # rows per tile
SUPER = 4                   # tiles per supertile
NSUPER = BLOC // (P * SUPER)  # 8
NPAIRS = D * (D + 1) // 2   # 528
CHUNKS = [128, 128, 128, 128, 16]   # pair rows per contraction chunk
CHUNK_P = [128, 128, 128, 128, 48]  # partitions per chunk (chunk4: +32 X rows)
KG = 34                     # k-grid width: 32 order-3 + out_low + zero pad
NCOL = KOUT * KG            # 1088 psum columns
XW = D + 2                  # host-padded X width: 32 + [1.0, 0.0]

PAIRS = [(i, j) for i in range(D) for j in range(i, D)]
I_P = np.array([p[0] for p in PAIRS], np.int32)
J_P = np.array([p[1] for p in PAIRS], np.int32)

F32 = mybir.dt.float32
BF = mybir.dt.bfloat16


def _pack_weights(W1, W2, W3):
    W1 = np.asarray(W1, np.float64)
    W2 = np.asarray(W2, np.float64)
    W3 = np.asarray(W3, np.float64)
    Wcat = np.zeros((5, 128, KOUT, KG), np.float64)
    for p, (i, j) in enumerate(PAIRS):
        c, pp = divmod(p, 128)
        if i < j:
            w3 = W3[i, j] + W3[j, i]   # [k, l]
            w2 = W2[i, j] + W2[j, i]   # [l]
        else:
            w3 = W3[i, i]
            w2 = W2[i, i]
        Wcat[c, pp, :, :D] = w3.T      # col l*34+k
        Wcat[c, pp, :, D] = w2         # out_low column
    for dd in range(D):                # order-1: X rows in chunk 4
        Wcat[4, 16 + dd, :, D] = W1[dd]
    Wcat = Wcat.reshape(5, 128, NCOL).astype(np.float32).astype(BF16)

    Sexp = np.zeros((5, 32, 128), np.float32)
    Srep = np.zeros((5, 32, 128), np.float32)
    off = 0
    for c, pc in enumerate(CHUNKS):
        for pp in range(pc):
            Sexp[c, I_P[off + pp], pp] = 1.0
            Srep[c, J_P[off + pp], pp] = 1.0
        off += pc
    return Wcat, Sexp.astype(BF16), Srep.astype(BF16)


def _build_module():
    nc = bacc.Bacc("TRN2", target_bir_lowering=False, debug=False,
                   enable_asserts=False)
    XBd = nc.dram_tensor("XB", [BLOC, XW], BF, kind="ExternalInput").ap()
    XTd = nc.dram_tensor("XT", [D, BLOC], BF, kind="ExternalInput").ap()
    WCd = nc.dram_tensor("WCAT", [5, 128, NCOL], BF, kind="ExternalInput").ap()
    SEd = nc.dram_tensor("SEXP", [5, 32, 128], BF, kind="ExternalInput").ap()
    SRd = nc.dram_tensor("SREP", [5, 32, 128], BF, kind="ExternalInput").ap()
    OUTd = nc.dram_tensor("OUT", [BLOC, KOUT], F32, kind="ExternalOutput").ap()

    with ExitStack() as ctx:
        tc = ctx.enter_context(tile.TileContext(nc))
        consts = ctx.enter_context(tc.tile_pool(name="consts", bufs=1))
        xbpool = ctx.enter_context(tc.tile_pool(name="xbpool", bufs=3 * SUPER))
        xtpool = ctx.enter_context(tc.tile_pool(name="xtpool", bufs=4))
        repsb = ctx.enter_context(tc.tile_pool(name="repsb", bufs=4))
        zpool = ctx.enter_context(tc.tile_pool(name="zpool", bufs=3))
        spool = ctx.enter_context(tc.tile_pool(name="spool", bufs=3))
        upool = ctx.enter_context(tc.tile_pool(name="upool", bufs=3))
        opool = ctx.enter_context(tc.tile_pool(name="opool", bufs=4))
        bps = ctx.enter_context(tc.tile_pool(name="bps", bufs=2, space="PSUM"))
        t3ps = ctx.enter_context(tc.tile_pool(name="t3ps", bufs=2, space="PSUM"))

        # load constants (small/critical first so they aren't queued behind W)
        se_sb, sr_sb = [], []
        for c in range(5):
            se = consts.tile([32, 128], BF, tag=f"se_{c}")
            nc.scalar.dma_start(out=se, in_=SEd[c])
            se_sb.append(se)
            sr = consts.tile([32, 128], BF, tag=f"sr_{c}")
            nc.scalar.dma_start(out=sr, in_=SRd[c])
            sr_sb.append(sr)
        w_sb = []
        for c in range(5):
            w = consts.tile([128, NCOL], BF, tag=f"w_{c}")
            nc.scalar.dma_start(out=w, in_=WCd[c])
            w_sb.append(w)

        def build(s):
            """DMA x tiles and build Z^T chunks for supertile s."""
            row0 = s * SUPER * P
            xt = xtpool.tile([D, SUPER * P], BF, tag="xt")
            nc.sync.dma_start(out=xt, in_=XTd[:, row0: row0 + SUPER * P])
            xbs = []
            for t in range(SUPER):
                xb = xbpool.tile([P, XW], BF, tag="xb")
                nc.sync.dma_start(out=xb, in_=XBd[row0 + t * P: row0 + (t + 1) * P, :])
                xbs.append(xb)
            zs = []
            for c, pc in enumerate(CHUNKS):
                exp_ps = bps.tile([128, SUPER * P], F32, tag="bps")
                rep_ps = bps.tile([128, SUPER * P], F32, tag="bps")
                nc.tensor.matmul(exp_ps[:pc], se_sb[c][:, :pc], xt[0:D, :],
                                 start=True, stop=True)
                nc.tensor.matmul(rep_ps[:pc], sr_sb[c][:, :pc], xt[0:D, :],
                                 start=True, stop=True)
                rep_s = repsb.tile([128, SUPER * P], F32, tag="repsb")
                nc.scalar.copy(out=rep_s[:pc], in_=rep_ps[:pc])
                z = zpool.tile([CHUNK_P[c], SUPER * P], BF, tag=f"z{c}")
                nc.vector.tensor_mul(z[:pc], exp_ps[:pc], rep_s[:pc])
                zs.append(z)
            # order-1 rows: X^T itself sits at partitions 16:48 of chunk 4
            nc.sync.dma_start(out=zs[4][16:48, :], in_=XTd[:, row0: row0 + SUPER * P])
            return xt, xbs, zs

        state = build(0)
        for s in range(NSUPER):
            xt, xbs, zs = state
            if s + 1 < NSUPER:
                state = build(s + 1)
            row0 = s * SUPER * P
            for t in range(SUPER):
                bsl = slice(t * P, (t + 1) * P)
                t3 = t3ps.tile([P, NCOL], F32, tag="t3")
                for c in range(5):
                    pcp = CHUNK_P[c]
                    first, last = c == 0, c == 4
                    for n0, n1 in ((0, 512), (512, 1024), (1024, NCOL)):
                        nc.tensor.matmul(t3[:, n0:n1], zs[c][:pcp, bsl],
                                         w_sb[c][:pcp, n0:n1],
                                         start=first, stop=last)
                staged = spool.tile([P, NCOL], BF, tag="staged")
                nc.scalar.copy(out=staged, in_=t3)
                u = upool.tile([P, NCOL], BF, tag="u")
                xk = xbs[t][:, :].unsqueeze(1).broadcast_to([P, KOUT, XW])
                nc.vector.tensor_mul(
                    u[:, :].rearrange("p (l k) -> p l k", k=KG),
                    staged[:, :].rearrange("p (l k) -> p l k", k=KG),
                    xk,
                )
                osb = opool.tile([P, KOUT], F32, tag="osb")
                nc.vector.reduce_sum(
                    out=osb, in_=u[:, :].rearrange("p (l k) -> p l k", k=KG),
                    axis=mybir.AxisListType.X,
                )
                nc.scalar.dma_start(out=OUTd[row0 + t * P: row0 + (t + 1) * P, :],
                                    in_=osb)
    nc.compile()
    return nc


_CACHE = {}


def _get_module():
    if "nc" not in _CACHE:
        _CACHE["nc"] = _build_module()
    return _CACHE["nc"]


def kernel(X, W1, W2, W3, bias):
    X = np.ascontiguousarray(np.asarray(X, np.float32))
    bias = np.asarray(bias, np.float32)
    Wcat, Sexp, Srep = _pack_weights(W1, W2, W3)

    nc = _get_module()
    Xb = X.astype(BF16)                      # [B, D] bf16 (single rounding point)
    XbT = np.ascontiguousarray(Xb.T)         # [D, B] bf16
    Xpad = np.zeros((B, XW), BF16)
    Xpad[:, :D] = Xb
    Xpad[:, D] = BF16(1.0)
    shards = Xpad.reshape(NCORES, BLOC, XW)
    in_maps = [
        {
            "XB": np.ascontiguousarray(shards[c]),
            "XT": np.ascontiguousarray(XbT[:, c * BLOC:(c + 1) * BLOC]),
            "WCAT": Wcat,
            "SEXP": Sexp,
            "SREP": Srep,
        }
        for c in range(NCORES)
    ]
    res = bass_utils.run_bass_kernel_spmd(nc, in_maps, core_ids=list(range(NCORES)))
    _CACHE["last_results"] = res
    out = np.concatenate([np.asarray(res.results[c]["OUT"]) for c in range(NCORES)], 0)
    return (out + bias.reshape(1, KOUT)).astype(np.float32)
